# revision 5
# baseline (speedup 1.0000x reference)
"""Single-head attention (B=4, S=4096, D=128), f32 in/out, on 8 TRN2 NeuronCores.

Sharding: data-parallel over (batch, query-half): core c handles batch c//2,
query rows (c%2)*2048 .. +2048. Weights replicated. Each core runs a
flash-style attention:
  - host pre-transposes x so d is on partitions (pure layout, done in numpy)
  - QKV projections on PE (f32 weights/activations; Q,K emitted as bf16
    with the 1/sqrt(128) scale folded into Q)
  - pass 1: bf16 Q@K^T scores in [q_part, k_free] chunks -> DVE row-max
  - pass 2: recompute scores -> ACT exp(psum - max) with fused per-partition
    bias, bf16 probs out, accum_out gives the softmax denominator for free
  - DMA-transpose probs (bf16 XBAR path) to [k_part, q_free]
  - PV on PE (bf16), then scale by 1/l and DMA out.

bf16 scores are safe here: measured rel_err vs f32 reference is ~2.6e-3
(the softmax is extremely peaked, but top-2 gaps are >1 for 95% of rows and
bf16 score error is ~0.3 absolute).
"""

import math
from contextlib import ExitStack

import numpy as np

import concourse.bass as bass
import concourse.tile as tile
from concourse import bacc, mybir
from concourse.bass_utils import run_bass_kernel_spmd

P = 128
D = 128
B = 4
S = 4096
N_CORES = 8
SQ = S * B // N_CORES  # 2048 query rows per core
SK = S  # keys per core (full batch sequence)
NQT = SQ // P  # 16 query tiles
NKT = SK // P  # 32 key tiles
KC = 512  # score chunk width (one PSUM bank)
NKC = SK // KC  # 8 chunks per query tile
SCALE = 1.0 / math.sqrt(D)

F32 = mybir.dt.float32
BF16 = mybir.dt.bfloat16


def build_bass() -> bacc.Bacc:
    nc = bacc.Bacc("TRN2", target_bir_lowering=False, debug=False)

    xqT = nc.declare_dram_parameter("xqT", [P, SQ], F32, isOutput=False)
    xkT = nc.declare_dram_parameter("xkT", [P, SK], F32, isOutput=False)
    wq = nc.declare_dram_parameter("wq", [D, D], F32, isOutput=False)
    wk = nc.declare_dram_parameter("wk", [D, D], F32, isOutput=False)
    wv = nc.declare_dram_parameter("wv", [D, D], F32, isOutput=False)
    out_ext = nc.declare_dram_parameter("out", [SQ, D], F32, isOutput=True)

    with tile.TileContext(nc) as tc, ExitStack() as ctx:
        const = ctx.enter_context(tc.tile_pool(name="const", bufs=1))
        psA = ctx.enter_context(tc.tile_pool(name="psA", bufs=3, space="PSUM"))
        psB = ctx.enter_context(tc.tile_pool(name="psB", bufs=3, space="PSUM"))
        pspv = ctx.enter_context(tc.tile_pool(name="pspv", bufs=2, space="PSUM"))
        probs_pool = ctx.enter_context(tc.tile_pool(name="probs", bufs=3))
        pT_pool = ctx.enter_context(tc.tile_pool(name="probsT", bufs=2))
        stat = ctx.enter_context(tc.tile_pool(name="stat", bufs=4))
        out_pool = ctx.enter_context(tc.tile_pool(name="outp", bufs=3))

        # ---- load inputs ----
        xqT_sb = const.tile([P, SQ], F32)
        nc.sync.dma_start(xqT_sb[:], xqT[:])
        xkT_sb = const.tile([P, SK], F32)
        nc.sync.dma_start(xkT_sb[:], xkT[:])
        wq_sb = const.tile([D, D], F32)
        nc.sync.dma_start(wq_sb[:], wq[:])
        wk_sb = const.tile([D, D], F32)
        nc.sync.dma_start(wk_sb[:], wk[:])
        wv_sb = const.tile([D, D], F32)
        nc.sync.dma_start(wv_sb[:], wv[:])

        # ---- projections ----
        # qbf[e, q] = sum_d wq[d, e] * xq[q, d] * SCALE   (bf16)
        qbf = const.tile([P, SQ], BF16)
        for i in range(SQ // 512):
            ps = psA.tile([P, 512], F32, tag="a")
            nc.tensor.matmul(
                ps[:],
                lhsT=wq_sb[:],
                rhs=xqT_sb[:, i * 512 : (i + 1) * 512],
                start=True,
                stop=True,
            )
            nc.scalar.activation(
                qbf[:, i * 512 : (i + 1) * 512],
                ps[:],
                mybir.ActivationFunctionType.Copy,
                scale=SCALE,
            )
        kbf = const.tile([P, SK], BF16)
        for i in range(SK // 512):
            ps = psA.tile([P, 512], F32, tag="a")
            nc.tensor.matmul(
                ps[:],
                lhsT=wk_sb[:],
                rhs=xkT_sb[:, i * 512 : (i + 1) * 512],
                start=True,
                stop=True,
            )
            nc.scalar.activation(
                kbf[:, i * 512 : (i + 1) * 512],
                ps[:],
                mybir.ActivationFunctionType.Copy,
            )
        # vbf[k_part, kt, d] = V[kt*128 + k_part, d]  (bf16)
        vbf = const.tile([P, NKT, D], BF16)
        for kt in range(NKT):
            ps = psB.tile([P, D], F32, tag="b")
            nc.tensor.matmul(
                ps[:],
                lhsT=xkT_sb[:, kt * P : (kt + 1) * P],
                rhs=wv_sb[:],
                start=True,
                stop=True,
            )
            nc.any.tensor_copy(out=vbf[:, kt, :], in_=ps[:])

        # ---- attention ----
        for qt in range(NQT):
            q_sl = qbf[:, qt * P : (qt + 1) * P]

            # pass 1: row maxes
            mx = stat.tile([P, NKC], F32, tag="mx")
            for c in range(NKC):
                ps = psA.tile([P, KC], F32, tag="a")
                nc.tensor.matmul(
                    ps[:],
                    lhsT=q_sl,
                    rhs=kbf[:, c * KC : (c + 1) * KC],
                    start=True,
                    stop=True,
                )
                nc.vector.reduce_max(
                    mx[:, c : c + 1], ps[:], axis=mybir.AxisListType.X
                )
            negm = stat.tile([P, 1], F32, tag="negm")
            nc.vector.tensor_reduce(
                negm[:], mx[:], axis=mybir.AxisListType.X,
                op=mybir.AluOpType.max, negate=True,
            )

            # pass 2: probs = exp(scores - max), bf16; accumulate row sums
            accs = stat.tile([P, NKC], F32, tag="accs")
            probs = probs_pool.tile([P, SK], BF16)
            for c in range(NKC):
                ps = psB.tile([P, KC], F32, tag="b")
                nc.tensor.matmul(
                    ps[:],
                    lhsT=q_sl,
                    rhs=kbf[:, c * KC : (c + 1) * KC],
                    start=True,
                    stop=True,
                )
                nc.scalar.activation(
                    probs[:, c * KC : (c + 1) * KC],
                    ps[:],
                    mybir.ActivationFunctionType.Exp,
                    bias=negm[:],
                    scale=1.0,
                    accum_out=accs[:, c : c + 1],
                )
            l_sum = stat.tile([P, 1], F32, tag="lsum")
            nc.vector.reduce_sum(l_sum[:], accs[:], axis=mybir.AxisListType.X)
            r_sb = stat.tile([P, 1], F32, tag="recip")
            nc.vector.reciprocal(r_sb[:], l_sum[:])

            # transpose probs -> [k_part, q]
            probsT = pT_pool.tile([P, NKT, P], BF16)
            nc.sync.dma_start_transpose(probsT[:], probs[:])

            # PV: psum[q, d] = sum_kt probsT[:, kt, :].T @ vbf[:, kt, :]
            po = pspv.tile([P, D], F32, tag="pv")
            for kt in range(NKT):
                nc.tensor.matmul(
                    po[:],
                    lhsT=probsT[:, kt, :],
                    rhs=vbf[:, kt, :],
                    start=(kt == 0),
                    stop=(kt == NKT - 1),
                )
            ot = out_pool.tile([P, D], F32, tag="ot")
            nc.vector.tensor_scalar_mul(ot[:], po[:], r_sb[:])
            nc.sync.dma_start(out_ext[qt * P : (qt + 1) * P, :], ot[:])

    nc.compile()
    return nc


_NC_CACHE: bacc.Bacc | None = None


def _get_nc() -> bacc.Bacc:
    global _NC_CACHE
    if _NC_CACHE is None:
        _NC_CACHE = build_bass()
    return _NC_CACHE


def kernel(**inputs: np.ndarray) -> np.ndarray:
    x = np.asarray(inputs["x"], dtype=np.float32)
    wq = np.ascontiguousarray(np.asarray(inputs["w_query"], dtype=np.float32))
    wk = np.ascontiguousarray(np.asarray(inputs["w_key"], dtype=np.float32))
    wv = np.ascontiguousarray(np.asarray(inputs["w_value"], dtype=np.float32))

    nc = _get_nc()

    in_maps = []
    for c in range(N_CORES):
        b = c // 2
        qoff = (c % 2) * SQ
        xT = np.ascontiguousarray(x[b].T)  # [128, 4096]
        xqT = np.ascontiguousarray(xT[:, qoff : qoff + SQ])  # [128, 2048]
        in_maps.append(
            {"xqT": xqT, "xkT": xT, "wq": wq, "wk": wk, "wv": wv}
        )

    res = run_bass_kernel_spmd(nc, in_maps, core_ids=list(range(N_CORES)))

    out = np.empty((B, S, D), dtype=np.float32)
    for c in range(N_CORES):
        b = c // 2
        qoff = (c % 2) * SQ
        out[b, qoff : qoff + SQ, :] = res.results[c]["out"]
    return out
